# revision 40
# baseline (speedup 1.0000x reference)
"""GPT-NeoX attention (B=4, S=1024, D=2048, H=16) on 8 TRN2 NeuronCores.

Tensor-parallel over heads: 2 heads per core. Each core computes its slice
of the fused QKV projection, RoPE, causal attention, and writes the
transposed per-head output [hd, S]; the host concatenates heads.

All matmul operands are bf16 (fp32 PSUM accumulation), which halves HBM
traffic and LDWEIGHTS time vs fp32r at identical PE stream rate. Layouts
avoid on-chip transposes:
  - x is fed transposed  xT[feature, token]
  - q,k are produced transposed  qT/kT[hd, token]  (RoPE applied in place)
  - v is produced natural  v[token, hd]  via a second projection pass
  - scores are computed transposed  sT[k_token, q_token]
  - out is produced transposed  oT[hd, q_token] = v.T @ expT
  - softmax sum over k = ones-vector matmul; normalization applied to oT
    via a reciprocal multiply of the replicated row-sum tile.
"""

import os

import ml_dtypes
import numpy as np

import concourse.bass as bass
import concourse.tile as tile
from concourse import bacc, mybir

# Problem constants (contract: nn_GPTNeoXAttention, fixed shapes)
B, S, D = 4, 1024, 2048
H = 16
HD = 128  # head dim
NCORES = 8
HPC = H // NCORES  # heads per core
ROPE_BASE = 10000.0
T = B * S  # 4096 tokens
KC = D // 128  # 16 contraction chunks of the model dim
NSL = 512  # token-slice width for the qk projection
NHALF = S // NSL  # 2 slices per batch
QCH = S // 512  # q slices per sequence in attention
SCALE = 1.0 / float(np.sqrt(HD))

F32 = mybir.dt.float32
BF16 = mybir.dt.bfloat16
NP_BF16 = ml_dtypes.bfloat16

_CACHE = {}


def _build_program():
    nc = bacc.Bacc(
        "TRN2", target_bir_lowering=False, debug=False, num_devices=NCORES
    )

    # per-half-batch contiguous layout: [128, B*NHALF, KC*NSL]
    x_d = nc.dram_tensor("x", [128, B * NHALF, KC * NSL], BF16, kind="ExternalInput")
    wqk_d = nc.dram_tensor("wqk", [128, 4, KC, 128], BF16, kind="ExternalInput")
    wv_d = nc.dram_tensor("wv", [128, KC, 2 * HD], BF16, kind="ExternalInput")
    bqk_d = nc.dram_tensor("bqk", [128, 4], F32, kind="ExternalInput")
    bv_d = nc.dram_tensor("bv", [128, 2 * HD], BF16, kind="ExternalInput")
    cos_d = nc.dram_tensor("cosT", [128, S], BF16, kind="ExternalInput")
    sin_d = nc.dram_tensor("sinS", [128, S], BF16, kind="ExternalInput")
    mask_d = nc.dram_tensor("masks", [128, 128], BF16, kind="ExternalInput")
    rot_d = nc.dram_tensor("rotT", [128, 128], BF16, kind="ExternalInput")
    ones_d = nc.dram_tensor("ones", [128, 128], BF16, kind="ExternalInput")
    out_d = nc.dram_tensor("out", [HPC, HD, B, S], BF16, kind="ExternalOutput")

    x_ap = x_d.ap()
    out_ap = out_d.ap()

    Exp = mybir.ActivationFunctionType.Exp
    Identity = mybir.ActivationFunctionType.Identity

    with tile.TileContext(nc) as tc:
        with (
            tc.tile_pool(name="singles", bufs=1) as singles,
            tc.tile_pool(name="xin", bufs=2) as xin_pool,
            tc.tile_pool(name="qk", bufs=8) as qk_pool,
            tc.tile_pool(name="vp", bufs=2) as v_pool,
            tc.tile_pool(name="expp", bufs=6) as exp_pool,
            tc.tile_pool(name="tmp", bufs=4) as tmp_pool,
            tc.tile_pool(name="outp", bufs=3) as out_pool,
            tc.tile_pool(name="rcp", bufs=2) as rcp_pool,
            # shared 4-deep ring for proj/rope/v psums AND attention scores
            tc.tile_pool(name="ps_work", bufs=4, space="PSUM") as ps_work,
            tc.tile_pool(name="ps_o", bufs=2, space="PSUM") as ps_o,
            tc.tile_pool(name="ps_sum", bufs=2, space="PSUM") as ps_sum,
        ):
            # First DMA wave holds only what gates the first m-chain
            # (wqk m0/m1, x(0,0), small constants). Everything else is
            # emitted behind compute-dependent queue positions so its
            # transfer doesn't steal bandwidth from the critical path.
            wqk_sb = singles.tile([128, 4, KC, 128], BF16)
            wv_sb = singles.tile([128, KC, 2 * HD], BF16)
            # small constants on the pool queue, earliest-needed first
            bqk_sb = singles.tile([128, 4], F32)
            nc.gpsimd.dma_start(out=bqk_sb, in_=bqk_d.ap())
            rot_sb = singles.tile([128, 128], BF16)
            nc.gpsimd.dma_start(out=rot_sb, in_=rot_d.ap())
            cos_sb = singles.tile([128, S], BF16)
            nc.gpsimd.dma_start(out=cos_sb, in_=cos_d.ap())
            sin_sb = singles.tile([128, S], BF16)
            nc.gpsimd.dma_start(out=sin_sb, in_=sin_d.ap())
            bv_sb = singles.tile([128, 2 * HD], BF16)
            nc.gpsimd.dma_start(out=bv_sb, in_=bv_d.ap())
            mask_sb = singles.tile([128, 128], BF16)
            # ones[128,128] lhsT: ones.T @ expT = sum over k, replicated
            # across all 128 output partitions (broadcast-ready layout)
            ones_sb = singles.tile([128, 128], BF16)

            x_tiles = {}

            def fetch_x(b, halves=(0, 1), eng=nc.sync):
                # flat tile [128, KC*NSL]; two DMAs per half so the first
                # m-chain can start after half the contraction chunks land
                for half in halves:
                    idx = b * NHALF + half
                    xsb = xin_pool.tile(
                        [128, KC * NSL], BF16, tag="x", name=f"x_{b}_{half}"
                    )
                    hn = KC * NSL // 2
                    eng.dma_start(out=xsb[:, :hn], in_=x_ap[:, idx, :hn])
                    eng.dma_start(out=xsb[:, hn:], in_=x_ap[:, idx, hn:])
                    x_tiles[(b, half)] = xsb

            for m in range(2):
                nc.scalar.dma_start(
                    out=wqk_sb[:, m, :, :], in_=wqk_d.ap()[:, m, :, :]
                )
            # batch 0 half 0 in four quarter DMAs on one queue: the
            # first m-chain starts when the first 4 kc-chunks land and
            # is then DMA-paced (the clock ramp hides inside the pacing)
            x00 = xin_pool.tile([128, KC * NSL], BF16, tag="x", name="x_0_0")
            x_tiles[(0, 0)] = x00
            qn = KC * NSL // 4
            for qi in range(4):
                nc.sync.dma_start(
                    out=x00[:, qi * qn : (qi + 1) * qn],
                    in_=x_ap[:, 0, qi * qn : (qi + 1) * qn],
                )

            # warm the PE clock/pipeline on zeros while the first DMA
            # wave is in flight (the first ~13 matmuls otherwise run at
            # ~0.6x clock); sized to end as x(0,0) lands
            scratch = singles.tile([128, 512], BF16)
            nc.vector.memzero(scratch)
            junk_ps = ps_work.tile([128, 512], F32, tag="ps")
            for _ in range(9):
                nc.tensor.matmul(
                    junk_ps, scratch[:, :128], scratch, start=True, stop=True
                )

            for b in range(B):
                # feature-major q/k tiles for this batch:
                # m=0: q head0, m=1: q head1, m=2: k head0, m=3: k head1
                qk_tiles = [
                    qk_pool.tile([128, S], BF16, tag="qkt", name=f"qkt_{b}_{i}")
                    for i in range(4)
                ]
                # natural-layout v for this batch: [token(128), chunk, 2*HD]
                v_sb = v_pool.tile([128, S // 128, 2 * HD], BF16)

                for half in range(NHALF):
                    xsb = x_tiles[(b, half)]
                    sl = slice(half * NSL, (half + 1) * NSL)
                    qbs = [None] * 4

                    def emit_rope(m, sl=sl, qbs=qbs, qk_tiles=qk_tiles):
                        # RoPE: rotate_half via PE permutation matmul, then
                        # same-partition elementwise combine on DVE. Emitted
                        # one m behind so the rot matmul never waits on ACT.
                        qb = qbs[m]
                        dst = qk_tiles[m][:, sl]
                        ps2 = ps_work.tile([128, NSL], F32, tag="ps")
                        nc.tensor.matmul(ps2, rot_sb, qb, start=True, stop=True)
                        tmp2 = tmp_pool.tile([128, NSL], BF16, tag="tmp2")
                        nc.vector.tensor_mul(tmp2, ps2, sin_sb[:, sl])
                        nc.vector.tensor_mul(dst, qb, cos_sb[:, sl])
                        nc.vector.tensor_add(dst, dst, tmp2)

                    # ---- q/k projection (transposed out: [feature, token]) ----
                    for m in range(4):
                        ps = ps_work.tile([128, NSL], F32, tag="ps")
                        for kc in range(KC):
                            nc.tensor.matmul(
                                ps,
                                wqk_sb[:, m, kc, :],
                                xsb[:, kc * NSL : kc * NSL + NSL],
                                start=(kc == 0),
                                stop=(kc == KC - 1),
                            )
                        # bias add (per-partition scalar) on ACT, PSUM -> SBUF
                        qb = tmp_pool.tile([128, NSL], BF16, tag="qb")
                        nc.scalar.activation(
                            qb, ps, Identity, bias=bqk_sb[:, m : m + 1], scale=1.0
                        )
                        qbs[m] = qb
                        if b == 0 and half == 0:
                            # loads not needed by the first m-chain, kept
                            # out of emission order's critical prefix
                            if m == 0:
                                nc.scalar.dma_start(
                                    out=wqk_sb[:, 2, :, :],
                                    in_=wqk_d.ap()[:, 2, :, :],
                                )
                                fetch_x(0, halves=(1,), eng=nc.scalar)
                                nc.scalar.dma_start(
                                    out=wqk_sb[:, 3, :, :],
                                    in_=wqk_d.ap()[:, 3, :, :],
                                )
                                nc.scalar.dma_start(out=wv_sb, in_=wv_d.ap())
                            elif m == 1:
                                nc.scalar.dma_start(
                                    out=mask_sb, in_=mask_d.ap()
                                )
                                nc.scalar.dma_start(
                                    out=ones_sb, in_=ones_d.ap()
                                )
                        if m >= 1:
                            emit_rope(m - 1)

                    # ---- v projection (natural out: [token, feature]) ----
                    for t in range(NSL // 128):
                        psv = ps_work.tile([128, 2 * HD], F32, tag="ps")
                        for kc in range(KC):
                            c0 = kc * NSL + t * 128
                            nc.tensor.matmul(
                                psv,
                                xsb[:, c0 : c0 + 128],
                                wv_sb[:, kc, :],
                                start=(kc == 0),
                                stop=(kc == KC - 1),
                            )
                        if t == 0:
                            emit_rope(3)
                        nc.vector.tensor_add(
                            v_sb[:, half * (NSL // 128) + t, :], psv, bv_sb
                        )

                # prefetch next batch's activations during attention
                if b + 1 < B:
                    fetch_x(b + 1)

                # ---- attention for this batch ----
                for h in range(HPC):
                    qT = qk_tiles[h]
                    kT = qk_tiles[2 + h]
                    for qs in range(QCH):
                        nk = (qs * 512 + 512) // 128  # causal: k chunks needed
                        ps_out = ps_o.tile([128, 512], F32)
                        ps_sm = ps_sum.tile([128, 512], F32)
                        qsl = slice(qs * 512, (qs + 1) * 512)
                        for ki in range(nk):
                            # causal narrowing: k-chunk ki only reaches
                            # queries q >= ki*128, so stream only those cols
                            off = max(0, ki * 128 - qs * 512)
                            cols = 512 - off
                            pss = ps_work.tile([128, 512], F32, tag="ps")
                            nc.tensor.matmul(
                                pss[:, :cols],
                                kT[:, ki * 128 : (ki + 1) * 128],
                                qT[:, qs * 512 + off : (qs + 1) * 512],
                                start=True,
                                stop=True,
                            )
                            e = exp_pool.tile([128, 512], BF16, tag="e")
                            nc.scalar.activation(
                                e[:, :cols], pss[:, :cols], Exp, scale=SCALE
                            )
                            if ki * 128 >= qs * 512:
                                # diagonal chunk: triangular boundary is
                                # always (local col >= partition)
                                nc.vector.tensor_mul(
                                    e[:, :128], e[:, :128], mask_sb
                                )
                            nc.tensor.matmul(
                                ps_out[:, off:],
                                v_sb[:, ki, h * HD : (h + 1) * HD],
                                e[:, :cols],
                                start=(ki == 0),
                                stop=(ki == nk - 1),
                            )
                            nc.tensor.matmul(
                                ps_sm[:, off:],
                                ones_sb,
                                e[:, :cols],
                                start=(ki == 0),
                                stop=(ki == nk - 1),
                            )
                        rc = rcp_pool.tile([128, 512], F32)
                        nc.vector.reciprocal_approx_fast(out=rc, in_=ps_sm)
                        o = out_pool.tile([128, 512], BF16)
                        nc.vector.tensor_mul(o, ps_out, rc)
                        # sync HWDGE queue: prefetch waits are resolved
                        # by emission time, so no head-of-line blocking
                        nc.sync.dma_start(
                            out=out_ap[h, :, b, qsl], in_=o
                        )

    nc.compile()
    return nc


def _prep_shared(hidden_states):
    x2 = np.ascontiguousarray(hidden_states.reshape(T, D).T)  # [D, T]
    # [128, KC, T] -> per-half-batch contiguous [128, B*NHALF, KC*NSL]
    x_host = np.ascontiguousarray(
        x2.reshape(KC, 128, B * NHALF, NSL).transpose(1, 2, 0, 3)
        .reshape(128, B * NHALF, KC * NSL)
    ).astype(NP_BF16)

    inv = 1.0 / (ROPE_BASE ** (np.arange(0, HD, 2, dtype=np.float64) / HD))
    f = np.outer(inv, np.arange(S, dtype=np.float64))  # [64, S]
    cosT = np.concatenate([np.cos(f), np.cos(f)], axis=0).astype(NP_BF16)
    sinS = np.concatenate([np.sin(f), np.sin(f)], axis=0).astype(NP_BF16)

    p = np.arange(128)[:, None]
    fcol = np.arange(128)[None, :]
    masks = np.ascontiguousarray((fcol >= p).astype(NP_BF16))  # [128, 128]

    # rotate_half as a matmul: out = lhsT.T @ rhs with lhsT = rotT gives
    # (R @ q)[i] = -q[i+64] (i<64), q[i-64] (i>=64)
    rotT = np.zeros((128, 128), NP_BF16)
    rotT[np.arange(64), np.arange(64) + 64] = 1.0
    rotT[np.arange(64) + 64, np.arange(64)] = -1.0
    return x_host, cosT, sinS, masks, rotT


def _core_rows(c):
    h0, h1 = 2 * c, 2 * c + 1
    rows = []
    for part in range(3):  # q, k, v blocks
        for h in (h0, h1):
            base = h * 3 * HD + part * HD
            rows.extend(range(base, base + HD))
    return np.asarray(rows)


def _prep_core(w_qkv, b_qkv, c):
    rows = _core_rows(c)
    wT = np.ascontiguousarray(w_qkv[rows, :].T)  # [D, 768]
    # qk features (4 m-blocks of 128), m-major layout [128, 4, KC, 128]
    wqk = np.ascontiguousarray(
        wT[:, : 4 * 128].reshape(KC, 128, 4, 128).transpose(1, 2, 0, 3)
    ).astype(NP_BF16)
    # v features, kc-major layout [128, KC, 256]
    wv = np.ascontiguousarray(
        wT[:, 4 * 128 :].reshape(KC, 128, 2 * HD).transpose(1, 0, 2)
    ).astype(NP_BF16)
    b_sel = b_qkv[rows]
    bqk = np.ascontiguousarray(
        b_sel[: 4 * 128].reshape(4, 128).T.astype(np.float32)
    )  # [128, 4]
    bv = np.ascontiguousarray(
        np.broadcast_to(b_sel[4 * 128 :].astype(NP_BF16), (128, 2 * HD))
    )  # [128, 256]
    return wqk, wv, bqk, bv


def _make_in_maps(hidden_states, w_qkv, b_qkv):
    x_host, cosT, sinS, masks, rotT = _prep_shared(hidden_states)
    in_maps = []
    for c in range(NCORES):
        wqk, wv, bqk, bv = _prep_core(w_qkv, b_qkv, c)
        in_maps.append(
            {
                "x": x_host,
                "wqk": wqk,
                "wv": wv,
                "bqk": bqk,
                "bv": bv,
                "cosT": cosT,
                "sinS": sinS,
                "masks": masks,
                "rotT": rotT,
                "ones": np.ones((128, 128), NP_BF16),
            }
        )
    return in_maps


def _assemble(results):
    outs = np.stack([results[c]["out"] for c in range(NCORES)])
    # [NCORES, HPC, HD, B, S] -> [B, S, H*HD]
    return np.ascontiguousarray(
        outs.reshape(H, HD, B, S).transpose(2, 3, 0, 1).reshape(B, S, D).astype(np.float32)
    )


def run(hidden_states, w_qkv, b_qkv, trace=False):
    from concourse.bass_utils import run_bass_kernel_spmd

    if "nc" not in _CACHE:
        _CACHE["nc"] = _build_program()
    nc = _CACHE["nc"]
    in_maps = _make_in_maps(
        np.asarray(hidden_states, dtype=np.float32),
        np.asarray(w_qkv, dtype=np.float32),
        np.asarray(b_qkv, dtype=np.float32),
    )
    res = run_bass_kernel_spmd(
        nc, in_maps, core_ids=list(range(NCORES)), trace=trace
    )
    out = _assemble(res.results)
    return out, res


def kernel(hidden_states, w_qkv, b_qkv):
    trace = os.environ.get("KERNEL_TRACE", "0") == "1"
    out, _res = run(hidden_states, w_qkv, b_qkv, trace=trace)
    return out


# revision 41
# speedup vs baseline: 1.0054x; 1.0054x over previous
"""GPT-NeoX attention (B=4, S=1024, D=2048, H=16) on 8 TRN2 NeuronCores.

Tensor-parallel over heads: 2 heads per core. Each core computes its slice
of the fused QKV projection, RoPE, causal attention, and writes the
transposed per-head output [hd, S]; the host concatenates heads.

All matmul operands are bf16 (fp32 PSUM accumulation), which halves HBM
traffic and LDWEIGHTS time vs fp32r at identical PE stream rate. Layouts
avoid on-chip transposes:
  - x is fed transposed  xT[feature, token]
  - q,k are produced transposed  qT/kT[hd, token]  (RoPE applied in place)
  - v is produced natural  v[token, hd]  via a second projection pass
  - scores are computed transposed  sT[k_token, q_token]
  - out is produced transposed  oT[hd, q_token] = v.T @ expT
  - softmax sum over k = ones-vector matmul; normalization applied to oT
    via a reciprocal multiply of the replicated row-sum tile.
"""

import os

import ml_dtypes
import numpy as np

import concourse.bass as bass
import concourse.tile as tile
from concourse import bacc, mybir

# Problem constants (contract: nn_GPTNeoXAttention, fixed shapes)
B, S, D = 4, 1024, 2048
H = 16
HD = 128  # head dim
NCORES = 8
HPC = H // NCORES  # heads per core
ROPE_BASE = 10000.0
T = B * S  # 4096 tokens
KC = D // 128  # 16 contraction chunks of the model dim
NSL = 512  # token-slice width for the qk projection
NHALF = S // NSL  # 2 slices per batch
QCH = S // 512  # q slices per sequence in attention
SCALE = 1.0 / float(np.sqrt(HD))

F32 = mybir.dt.float32
BF16 = mybir.dt.bfloat16
NP_BF16 = ml_dtypes.bfloat16

_CACHE = {}


def _build_program():
    nc = bacc.Bacc(
        "TRN2", target_bir_lowering=False, debug=False, num_devices=NCORES
    )

    # per-half-batch contiguous layout: [128, B*NHALF, KC*NSL]
    x_d = nc.dram_tensor("x", [128, B * NHALF, KC * NSL], BF16, kind="ExternalInput")
    wqk_d = nc.dram_tensor("wqk", [128, 4, KC, 128], BF16, kind="ExternalInput")
    wv_d = nc.dram_tensor("wv", [128, KC, 2 * HD], BF16, kind="ExternalInput")
    bqk_d = nc.dram_tensor("bqk", [128, 4], F32, kind="ExternalInput")
    bv_d = nc.dram_tensor("bv", [128, 2 * HD], BF16, kind="ExternalInput")
    cos_d = nc.dram_tensor("cosT", [128, S], BF16, kind="ExternalInput")
    sin_d = nc.dram_tensor("sinS", [128, S], BF16, kind="ExternalInput")
    mask_d = nc.dram_tensor("masks", [128, 128], BF16, kind="ExternalInput")
    rot_d = nc.dram_tensor("rotT", [128, 128], BF16, kind="ExternalInput")
    ones_d = nc.dram_tensor("ones", [128, 128], BF16, kind="ExternalInput")
    out_d = nc.dram_tensor("out", [HPC, HD, B, S], BF16, kind="ExternalOutput")

    x_ap = x_d.ap()
    out_ap = out_d.ap()

    Exp = mybir.ActivationFunctionType.Exp
    Identity = mybir.ActivationFunctionType.Identity

    with tile.TileContext(nc) as tc:
        with (
            tc.tile_pool(name="singles", bufs=1) as singles,
            tc.tile_pool(name="xin", bufs=2) as xin_pool,
            tc.tile_pool(name="qk", bufs=8) as qk_pool,
            tc.tile_pool(name="vp", bufs=2) as v_pool,
            tc.tile_pool(name="expp", bufs=6) as exp_pool,
            tc.tile_pool(name="tmp", bufs=4) as tmp_pool,
            tc.tile_pool(name="outp", bufs=3) as out_pool,
            tc.tile_pool(name="rcp", bufs=2) as rcp_pool,
            # shared 4-deep ring for proj/rope/v psums AND attention scores
            tc.tile_pool(name="ps_work", bufs=4, space="PSUM") as ps_work,
            tc.tile_pool(name="ps_o", bufs=2, space="PSUM") as ps_o,
            tc.tile_pool(name="ps_sum", bufs=2, space="PSUM") as ps_sum,
        ):
            # First DMA wave holds only what gates the first m-chain
            # (wqk m0/m1, x(0,0), small constants). Everything else is
            # emitted behind compute-dependent queue positions so its
            # transfer doesn't steal bandwidth from the critical path.
            wqk_sb = singles.tile([128, 4, KC, 128], BF16)
            wv_sb = singles.tile([128, KC, 2 * HD], BF16)
            # small constants on the pool queue, earliest-needed first
            bqk_sb = singles.tile([128, 4], F32)
            nc.gpsimd.dma_start(out=bqk_sb, in_=bqk_d.ap())
            rot_sb = singles.tile([128, 128], BF16)
            nc.gpsimd.dma_start(out=rot_sb, in_=rot_d.ap())
            cos_sb = singles.tile([128, S], BF16)
            nc.gpsimd.dma_start(out=cos_sb, in_=cos_d.ap())
            sin_sb = singles.tile([128, S], BF16)
            nc.gpsimd.dma_start(out=sin_sb, in_=sin_d.ap())
            bv_sb = singles.tile([128, 2 * HD], BF16)
            nc.gpsimd.dma_start(out=bv_sb, in_=bv_d.ap())
            mask_sb = singles.tile([128, 128], BF16)
            # ones[128,128] lhsT: ones.T @ expT = sum over k, replicated
            # across all 128 output partitions (broadcast-ready layout)
            ones_sb = singles.tile([128, 128], BF16)

            x_tiles = {}

            def fetch_x(b, halves=(0, 1), eng=nc.sync):
                # flat tile [128, KC*NSL]; two DMAs per half so the first
                # m-chain can start after half the contraction chunks land
                for half in halves:
                    idx = b * NHALF + half
                    xsb = xin_pool.tile(
                        [128, KC * NSL], BF16, tag="x", name=f"x_{b}_{half}"
                    )
                    hn = KC * NSL // 2
                    eng.dma_start(out=xsb[:, :hn], in_=x_ap[:, idx, :hn])
                    eng.dma_start(out=xsb[:, hn:], in_=x_ap[:, idx, hn:])
                    x_tiles[(b, half)] = xsb

            for m in range(2):
                nc.scalar.dma_start(
                    out=wqk_sb[:, m, :, :], in_=wqk_d.ap()[:, m, :, :]
                )
            fetch_x(0, halves=(0,))

            # warm the PE clock/pipeline on zeros while the first DMA
            # wave is in flight (the first ~13 matmuls otherwise run at
            # ~0.6x clock); sized to end as x(0,0) lands
            scratch = singles.tile([128, 512], BF16)
            nc.vector.memzero(scratch)
            junk_ps = ps_work.tile([128, 512], F32, tag="ps")
            for _ in range(27):
                nc.tensor.matmul(
                    junk_ps, scratch[:, :128], scratch, start=True, stop=True
                )

            for b in range(B):
                # feature-major q/k tiles for this batch:
                # m=0: q head0, m=1: q head1, m=2: k head0, m=3: k head1
                qk_tiles = [
                    qk_pool.tile([128, S], BF16, tag="qkt", name=f"qkt_{b}_{i}")
                    for i in range(4)
                ]
                # natural-layout v for this batch: [token(128), chunk, 2*HD]
                v_sb = v_pool.tile([128, S // 128, 2 * HD], BF16)

                for half in range(NHALF):
                    xsb = x_tiles[(b, half)]
                    sl = slice(half * NSL, (half + 1) * NSL)
                    qbs = [None] * 4

                    def emit_rope(m, sl=sl, qbs=qbs, qk_tiles=qk_tiles):
                        # RoPE: rotate_half via PE permutation matmul, then
                        # same-partition elementwise combine on DVE. Emitted
                        # one m behind so the rot matmul never waits on ACT.
                        qb = qbs[m]
                        dst = qk_tiles[m][:, sl]
                        ps2 = ps_work.tile([128, NSL], F32, tag="ps")
                        nc.tensor.matmul(ps2, rot_sb, qb, start=True, stop=True)
                        tmp2 = tmp_pool.tile([128, NSL], BF16, tag="tmp2")
                        nc.vector.tensor_mul(tmp2, ps2, sin_sb[:, sl])
                        nc.vector.tensor_mul(dst, qb, cos_sb[:, sl])
                        nc.vector.tensor_add(dst, dst, tmp2)

                    # ---- q/k projection (transposed out: [feature, token]) ----
                    for m in range(4):
                        ps = ps_work.tile([128, NSL], F32, tag="ps")
                        for kc in range(KC):
                            nc.tensor.matmul(
                                ps,
                                wqk_sb[:, m, kc, :],
                                xsb[:, kc * NSL : kc * NSL + NSL],
                                start=(kc == 0),
                                stop=(kc == KC - 1),
                            )
                        # bias add (per-partition scalar) on ACT, PSUM -> SBUF
                        qb = tmp_pool.tile([128, NSL], BF16, tag="qb")
                        nc.scalar.activation(
                            qb, ps, Identity, bias=bqk_sb[:, m : m + 1], scale=1.0
                        )
                        qbs[m] = qb
                        if b == 0 and half == 0:
                            # loads not needed by the first m-chain, kept
                            # out of emission order's critical prefix
                            if m == 0:
                                nc.scalar.dma_start(
                                    out=wqk_sb[:, 2, :, :],
                                    in_=wqk_d.ap()[:, 2, :, :],
                                )
                                fetch_x(0, halves=(1,), eng=nc.scalar)
                                nc.scalar.dma_start(
                                    out=wqk_sb[:, 3, :, :],
                                    in_=wqk_d.ap()[:, 3, :, :],
                                )
                                nc.scalar.dma_start(out=wv_sb, in_=wv_d.ap())
                            elif m == 1:
                                nc.scalar.dma_start(
                                    out=mask_sb, in_=mask_d.ap()
                                )
                                nc.scalar.dma_start(
                                    out=ones_sb, in_=ones_d.ap()
                                )
                        if m >= 1:
                            emit_rope(m - 1)

                    # ---- v projection (natural out: [token, feature]) ----
                    for t in range(NSL // 128):
                        psv = ps_work.tile([128, 2 * HD], F32, tag="ps")
                        for kc in range(KC):
                            c0 = kc * NSL + t * 128
                            nc.tensor.matmul(
                                psv,
                                xsb[:, c0 : c0 + 128],
                                wv_sb[:, kc, :],
                                start=(kc == 0),
                                stop=(kc == KC - 1),
                            )
                        if t == 0:
                            emit_rope(3)
                        nc.vector.tensor_add(
                            v_sb[:, half * (NSL // 128) + t, :], psv, bv_sb
                        )

                # prefetch next batch's activations during attention
                if b + 1 < B:
                    fetch_x(b + 1)

                # ---- attention for this batch ----
                for h in range(HPC):
                    qT = qk_tiles[h]
                    kT = qk_tiles[2 + h]
                    for qs in range(QCH):
                        nk = (qs * 512 + 512) // 128  # causal: k chunks needed
                        ps_out = ps_o.tile([128, 512], F32)
                        ps_sm = ps_sum.tile([128, 512], F32)
                        qsl = slice(qs * 512, (qs + 1) * 512)
                        for ki in range(nk):
                            # causal narrowing: k-chunk ki only reaches
                            # queries q >= ki*128, so stream only those cols
                            off = max(0, ki * 128 - qs * 512)
                            cols = 512 - off
                            pss = ps_work.tile([128, 512], F32, tag="ps")
                            nc.tensor.matmul(
                                pss[:, :cols],
                                kT[:, ki * 128 : (ki + 1) * 128],
                                qT[:, qs * 512 + off : (qs + 1) * 512],
                                start=True,
                                stop=True,
                            )
                            e = exp_pool.tile([128, 512], BF16, tag="e")
                            nc.scalar.activation(
                                e[:, :cols], pss[:, :cols], Exp, scale=SCALE
                            )
                            if ki * 128 >= qs * 512:
                                # diagonal chunk: triangular boundary is
                                # always (local col >= partition)
                                nc.vector.tensor_mul(
                                    e[:, :128], e[:, :128], mask_sb
                                )
                            nc.tensor.matmul(
                                ps_out[:, off:],
                                v_sb[:, ki, h * HD : (h + 1) * HD],
                                e[:, :cols],
                                start=(ki == 0),
                                stop=(ki == nk - 1),
                            )
                            nc.tensor.matmul(
                                ps_sm[:, off:],
                                ones_sb,
                                e[:, :cols],
                                start=(ki == 0),
                                stop=(ki == nk - 1),
                            )
                        rc = rcp_pool.tile([128, 512], F32)
                        nc.vector.reciprocal_approx_fast(out=rc, in_=ps_sm)
                        o = out_pool.tile([128, 512], BF16)
                        nc.vector.tensor_mul(o, ps_out, rc)
                        # sync HWDGE queue: prefetch waits are resolved
                        # by emission time, so no head-of-line blocking
                        nc.sync.dma_start(
                            out=out_ap[h, :, b, qsl], in_=o
                        )

    nc.compile()
    return nc


def _prep_shared(hidden_states):
    x2 = np.ascontiguousarray(hidden_states.reshape(T, D).T)  # [D, T]
    # [128, KC, T] -> per-half-batch contiguous [128, B*NHALF, KC*NSL]
    x_host = np.ascontiguousarray(
        x2.reshape(KC, 128, B * NHALF, NSL).transpose(1, 2, 0, 3)
        .reshape(128, B * NHALF, KC * NSL)
    ).astype(NP_BF16)

    inv = 1.0 / (ROPE_BASE ** (np.arange(0, HD, 2, dtype=np.float64) / HD))
    f = np.outer(inv, np.arange(S, dtype=np.float64))  # [64, S]
    cosT = np.concatenate([np.cos(f), np.cos(f)], axis=0).astype(NP_BF16)
    sinS = np.concatenate([np.sin(f), np.sin(f)], axis=0).astype(NP_BF16)

    p = np.arange(128)[:, None]
    fcol = np.arange(128)[None, :]
    masks = np.ascontiguousarray((fcol >= p).astype(NP_BF16))  # [128, 128]

    # rotate_half as a matmul: out = lhsT.T @ rhs with lhsT = rotT gives
    # (R @ q)[i] = -q[i+64] (i<64), q[i-64] (i>=64)
    rotT = np.zeros((128, 128), NP_BF16)
    rotT[np.arange(64), np.arange(64) + 64] = 1.0
    rotT[np.arange(64) + 64, np.arange(64)] = -1.0
    return x_host, cosT, sinS, masks, rotT


def _core_rows(c):
    h0, h1 = 2 * c, 2 * c + 1
    rows = []
    for part in range(3):  # q, k, v blocks
        for h in (h0, h1):
            base = h * 3 * HD + part * HD
            rows.extend(range(base, base + HD))
    return np.asarray(rows)


def _prep_core(w_qkv, b_qkv, c):
    rows = _core_rows(c)
    wT = np.ascontiguousarray(w_qkv[rows, :].T)  # [D, 768]
    # qk features (4 m-blocks of 128), m-major layout [128, 4, KC, 128]
    wqk = np.ascontiguousarray(
        wT[:, : 4 * 128].reshape(KC, 128, 4, 128).transpose(1, 2, 0, 3)
    ).astype(NP_BF16)
    # v features, kc-major layout [128, KC, 256]
    wv = np.ascontiguousarray(
        wT[:, 4 * 128 :].reshape(KC, 128, 2 * HD).transpose(1, 0, 2)
    ).astype(NP_BF16)
    b_sel = b_qkv[rows]
    bqk = np.ascontiguousarray(
        b_sel[: 4 * 128].reshape(4, 128).T.astype(np.float32)
    )  # [128, 4]
    bv = np.ascontiguousarray(
        np.broadcast_to(b_sel[4 * 128 :].astype(NP_BF16), (128, 2 * HD))
    )  # [128, 256]
    return wqk, wv, bqk, bv


def _make_in_maps(hidden_states, w_qkv, b_qkv):
    x_host, cosT, sinS, masks, rotT = _prep_shared(hidden_states)
    in_maps = []
    for c in range(NCORES):
        wqk, wv, bqk, bv = _prep_core(w_qkv, b_qkv, c)
        in_maps.append(
            {
                "x": x_host,
                "wqk": wqk,
                "wv": wv,
                "bqk": bqk,
                "bv": bv,
                "cosT": cosT,
                "sinS": sinS,
                "masks": masks,
                "rotT": rotT,
                "ones": np.ones((128, 128), NP_BF16),
            }
        )
    return in_maps


def _assemble(results):
    outs = np.stack([results[c]["out"] for c in range(NCORES)])
    # [NCORES, HPC, HD, B, S] -> [B, S, H*HD]
    return np.ascontiguousarray(
        outs.reshape(H, HD, B, S).transpose(2, 3, 0, 1).reshape(B, S, D).astype(np.float32)
    )


def run(hidden_states, w_qkv, b_qkv, trace=False):
    from concourse.bass_utils import run_bass_kernel_spmd

    if "nc" not in _CACHE:
        _CACHE["nc"] = _build_program()
    nc = _CACHE["nc"]
    in_maps = _make_in_maps(
        np.asarray(hidden_states, dtype=np.float32),
        np.asarray(w_qkv, dtype=np.float32),
        np.asarray(b_qkv, dtype=np.float32),
    )
    res = run_bass_kernel_spmd(
        nc, in_maps, core_ids=list(range(NCORES)), trace=trace
    )
    out = _assemble(res.results)
    return out, res


def kernel(hidden_states, w_qkv, b_qkv):
    trace = os.environ.get("KERNEL_TRACE", "0") == "1"
    out, _res = run(hidden_states, w_qkv, b_qkv, trace=trace)
    return out
